# revision 17
# baseline (speedup 1.0000x reference)
"""Trainium2 Bass kernel for nn_EnhancedGraphConv (gnn_message_passing).

Strategy (8 cores): shard the B*N=1280 graph rows (b,i) as 160 rows/core
(cores 0-3 -> batch 0, 4-7 -> batch 1).  The host sorts each core's rows by
degree and compacts the ~5% active edges into a padded r-major token stream
with a per-group slot count D_g = max degree in that 32-row group (the
padded token count drops ~28% vs a uniform max-degree pad).  Tokens are
pre-gathered and feature-major so the device does zero indirect DMA and
zero layout transposes.  All per-edge MLPs run as bf16 matmuls over
8-row chunks; sigmoid is computed as (1+tanh(x/2))/2 so every scalar-engine
function lives in one act-func set; softmax skips max-subtraction (scores
are O(1)); the adjacency mask enters as an extra contraction row of the
score matmul; exp(score) is broadcast across partitions by an SBUF-to-SBUF
DMA and the message weighting runs on the otherwise-idle gpsimd engine;
per-row reductions are a single windowed avg-pool over a 65-row tile whose
last row carries exp(score), so the softmax denominator falls out of the
same instruction.  The score/exp/broadcast tail of each chunk is
software-pipelined one chunk behind the matmul front.
"""
import numpy as np
from contextlib import ExitStack

import concourse.bass as bass
import concourse.bacc as bacc
import concourse.tile as tile
from concourse import mybir
from concourse.bass_utils import run_bass_kernel_spmd
from concourse.masks import make_identity

F32 = mybir.dt.float32
BF16 = mybir.dt.bfloat16
AF = mybir.ActivationFunctionType
OP = mybir.AluOpType
NPBF = mybir.dt.np(BF16)

B, N, C, O, E = 2, 640, 64, 64, 18
RG = 32           # rows per group
RCH = 8           # rows per chunk
NCORES = 8
RPC = (B * N) // NCORES   # 160 rows per core
NG = RPC // RG            # 5 groups
NCH = RG // RCH           # 4 chunks per group
NCHT = NG * NCH           # 20 chunks

NEG = -1e38

_DS = None        # per-group neighbor slot counts, set by _host_prep
_NC = {}          # compiled kernels keyed by (DS, repeat)


def _build_nc(repeat=1):
    DS = _DS
    RCs = [16 if d <= 32 else RCH for d in DS]
    CHs = [rc * d for rc, d in zip(RCs, DS)]
    TGs = [RG * d for d in DS]
    toff = np.concatenate([[0], np.cumsum(TGs)]).astype(int)
    roff = np.concatenate([[0], np.cumsum(TGs)]).astype(int)
    T = int(toff[-1])
    CHmax = max(CHs)
    chunks = []
    for g in range(NG):
        for q in range(RG // RCs[g]):
            chunks.append((g, q))
    NK = len(chunks)

    nc = bacc.Bacc("TRN2", target_bir_lowering=False)
    t = {}
    bf_inp = [
        ("tok", [82, T]), ("amp", [NCHT, CHmax]), ("R32", [64, T]),
        ("We1", [E, 64]), ("We2", [64, 64]), ("We3", [64, 32]),
        ("Wpe", [32, 128]), ("Wjj", [64, 128]), ("Wn2", [64, 64]),
        ("bnT", [1, 64]), ("W22", [128, 128]), ("Wa3a", [33, 1]),
    ]
    f_inp = [
        ("xrows", [RPC, C]),
        ("Wxi", [64, 64]), ("Ws", [64, 64]), ("Wc1", [128, 64]),
        ("Wc2", [64, 64]),
        ("be1", [64, 1]), ("be2", [64, 1]), ("be3", [32, 1]),
        ("ba2", [32, 1]), ("bg2h", [64, 1]), ("bhg", [128, 1]),
        ("bs", [64, 1]), ("bc1", [64, 1]), ("bc2", [64, 1]),
    ]
    for name, shape in bf_inp:
        t[name] = nc.dram_tensor(name, shape, BF16, kind="ExternalInput")
    for name, shape in f_inp:
        t[name] = nc.dram_tensor(name, shape, F32, kind="ExternalInput")
    t["out"] = nc.dram_tensor("out", [RPC, O], F32, kind="ExternalOutput")

    with tile.TileContext(nc) as tc, ExitStack() as ctx:
        w = ctx.enter_context(tc.tile_pool(name="w", bufs=1))
        chk = ctx.enter_context(tc.tile_pool(name="chk", bufs=4))
        grp = ctx.enter_context(tc.tile_pool(name="grp", bufs=2))
        psc = ctx.enter_context(tc.tile_pool(name="psc", bufs=1, space="PSUM"))
        pss = ctx.enter_context(tc.tile_pool(name="pss", bufs=1, space="PSUM"))

        for _rep in range(repeat):

            ident = w.tile([128, 128], F32)
            make_identity(nc, ident[:])
            ones64h = w.tile([128, 64], BF16)
            nc.vector.memset(ones64h[64:65, :], 1.0)
            onesCH = w.tile([1, CHmax], BF16)
            nc.vector.memset(onesCH[:], 1.0)
            # small loads on the gpsimd DGE queue, streaming tok/R32/xrows
            # on SP, roughly in first-use order
            worder = ["We1", "be1", "Wjj", "Wxi", "We2", "be2",
                      "Wn2", "bnT", "We3", "be3", "Wpe", "bhg", "W22", "ba2",
                      "bg2h", "Wa3a", "Ws", "bs", "Wc1", "bc1", "Wc2", "bc2"]
            wt = {}
            wtE1 = w.tile([128, 64], BF16)
            xr = w.tile([128, 2, C], F32)
            for name in worder:
                if name == "We1":
                    nc.gpsimd.dma_start(out=wtE1[64:64 + E, :], in_=t["We1"][:])
                    continue
                shape = dict(bf_inp + f_inp)[name]
                wt[name] = w.tile(shape, BF16 if dict(bf_inp).get(name) else F32,
                                  name=name)
                nc.gpsimd.dma_start(out=wt[name][:], in_=t[name][:])

            toks0 = []
            for q in range(NCH):
                t0q = w.tile([82, CHs[0]], BF16, name=f"tok0_{q}")
                nc.sync.dma_start(
                    out=t0q[:], in_=t["tok"][:, q * CHs[0]:(q + 1) * CHs[0]])
                toks0.append(t0q)
            nc.sync.dma_start(out=xr[:, 0, :], in_=t["xrows"][0:128, :])
            nc.sync.dma_start(out=xr[:32, 1, :], in_=t["xrows"][128:160, :])
            r32t = w.tile([64, T], BF16)
            nc.sync.dma_start(out=r32t[:], in_=t["R32"][:])
            toks = {0: None}
            tg1 = w.tile([82, TGs[1]], BF16, name="tokg_1")
            nc.sync.dma_start(out=tg1[:],
                              in_=t["tok"][:, int(toff[1]):int(toff[2])])
            toks[1] = tg1

            xrf = w.tile([C, RPC], F32)
            axiT = w.tile([64, 3, C], BF16)
            selff = w.tile([C, RPC], F32)

            def emit_axi_setup():
                p0 = pss.tile([C, RPC], F32, name="pgrp")
                nc.tensor.transpose(p0[:, :128], xr[:, 0, :], ident[:])
                nc.tensor.transpose(p0[:, 128:160], xr[:32, 1, :],
                                    ident[:32, :32])
                nc.vector.tensor_copy(out=xrf[:], in_=p0[:, :RPC])
                pa = pss.tile([C, RPC], F32, name="pgrp")
                nc.tensor.matmul(pa[:], wt["Wxi"][:], xrf[:],
                                 start=True, stop=True)
                axs = w.tile([C, RPC], F32)
                nc.vector.tensor_copy(out=axs[:], in_=pa[:])
                for blk, (c0, c1) in enumerate([(0, 64), (64, 128), (128, 160)]):
                    ptb = pss.tile([64, C], F32, name="pgrp")
                    nc.tensor.transpose(ptb[:c1 - c0, :], axs[:, c0:c1],
                                        ident[:64, :64])
                    nc.vector.tensor_copy(out=axiT[:c1 - c0, blk, :],
                                          in_=ptb[:c1 - c0, :])

            def emit_self_setup():
                pb = pss.tile([C, RPC], F32, name="pgrp")
                nc.tensor.matmul(pb[:], wt["Ws"][:], xrf[:],
                                 start=True, stop=True)
                nc.scalar.activation(selff[:], pb[:], AF.Identity,
                                     bias=wt["bs"][:])

            emit_axi_setup()
            emit_self_setup()

            msums = {}
            prev = None
            for k in range(NK + 1):
                # ---- tail A of previous chunk: score matmul + exp + bcast
                if prev is not None:
                    ph2a, pgm, pg, pq = prev
                    pCH = CHs[pg]
                    pD = DS[pg]
                    pRC = RCs[pg]
                    ps7 = pss.tile([65, CHmax], F32, name="ps7")
                    nc.tensor.matmul(ps7[64:65, :pCH], wt["Wa3a"][:], ph2a[:],
                                     start=True, stop=True)
                    pmdw = chk.tile([65, pCH], BF16, name="mdw",
                                    padded_shape=[65, CHmax])
                    nc.scalar.activation(pmdw[64:65, :], ps7[64:65, :pCH],
                                         AF.Exp)
                    if k < NK:
                        pexpb = chk.tile([64, pCH], BF16, name="pexpb",
                                         padded_shape=[64, CHmax])
                        srow = pmdw[64:65, :]
                        bsrc = bass.AP(tensor=pmdw.tensor, offset=srow.offset,
                                       ap=[srow.ap[0], [0, 64], [1, pCH]])
                        nc.sync.dma_start(out=pexpb[:], in_=bsrc)
                    else:
                        nc.tensor.matmul(ps7[0:64, :pCH], ones64h[64:65, :],
                                         pmdw[64:65, :], start=True, stop=True)

                # ---- front of current chunk
                if k < NK:
                    g, q = chunks[k]
                    D = DS[g]
                    CH = CHs[g]
                    RC = RCs[g]
                    if q == 0:
                        msums[g] = grp.tile([65, RG], F32, name="msum")
                    if g == 0:
                        tkt, c0 = toks0[q], 0
                    else:
                        tkt, c0 = toks[g], q * CH
                    cols = slice(c0, c0 + CH)
                    rb = g * RG + q * RC
                    blk = rb // 64
                    b32 = 32 * ((rb % 64) // 32)
                    r32c = int(roff[g]) + q * CH

                    ps1 = psc.tile([64, CHmax], F32, name="ps1")
                    nc.tensor.matmul(ps1[:, :CH], wtE1[64:64 + E, :],
                                     tkt[64:64 + E, cols], start=True, stop=True)
                    ps4 = psc.tile([128, CHmax], F32, name="ps4")
                    nc.tensor.matmul(ps4[:, :CH], wt["Wjj"][:], tkt[0:64, cols],
                                     start=True, stop=False)
                    nc.tensor.matmul(ps4[:64, :CH], axiT[b32:b32 + 32, blk, :],
                                     r32t[b32:b32 + 32, r32c:r32c + CH],
                                     start=False, stop=False)
                    ps5 = psc.tile([64, CHmax], F32, name="ps5")
                    nc.tensor.matmul(ps5[:, :CH], wt["Wn2"][:], tkt[0:64, cols],
                                     start=True, stop=False)
                    nc.tensor.matmul(ps5[:, :CH], wt["bnT"][:], onesCH[:, :CH],
                                     start=False, stop=True)
                    pe1 = chk.tile([64, CH], BF16, name="pe1",
                                   padded_shape=[64, CHmax])
                    nc.vector.tensor_scalar(out=pe1[:], in0=ps1[:, :CH],
                                            scalar1=wt["be1"][:], scalar2=0.0,
                                            op0=OP.add, op1=OP.max)
                    ps2 = psc.tile([64, CHmax], F32, name="ps2")
                    nc.tensor.matmul(ps2[:, :CH], wt["We2"][:], pe1[:],
                                     start=True, stop=True)
                    pe2 = chk.tile([64, CH], BF16, name="pe2",
                                   padded_shape=[64, CHmax])
                    nc.scalar.activation(pe2[:], ps2[:, :CH], AF.Relu,
                                         bias=wt["be2"][:])
                    ps3 = psc.tile([32, CHmax], F32, name="ps3")
                    nc.tensor.matmul(ps3[:, :CH], wt["We3"][:], pe2[:],
                                     start=True, stop=True)
                    pe3 = chk.tile([32, CH], BF16, name="pe3",
                                   padded_shape=[32, CHmax])
                    if k % 2 == 1:
                        nc.vector.tensor_scalar(out=pe3[:], in0=ps3[:, :CH],
                                                scalar1=wt["be3"][:],
                                                scalar2=0.0,
                                                op0=OP.add, op1=OP.max)
                    else:
                        nc.scalar.activation(pe3[:], ps3[:, :CH], AF.Relu,
                                             bias=wt["be3"][:])
                    nc.tensor.matmul(ps4[:, :CH], wt["Wpe"][:], pe3[:],
                                     start=False, stop=True)
                    hg = chk.tile([128, CH], BF16, name="hg",
                                  padded_shape=[128, CHmax])
                    nc.scalar.activation(hg[:], ps4[:, :CH], AF.Relu,
                                         bias=wt["bhg"][:])
                    ps6 = psc.tile([128, CHmax], F32, name="ps6")
                    nc.tensor.matmul(ps6[:, :CH], wt["W22"][:], hg[:],
                                     start=True, stop=True)
                    h2a = chk.tile([33, CH], BF16, name="h2a",
                                   padded_shape=[33, CHmax])
                    nc.sync.dma_start(out=h2a[32:33, :],
                                      in_=t["amp"][k:k + 1, 0:CH])
                    nc.vector.tensor_scalar(out=h2a[:32, :], in0=ps6[:32, :CH],
                                            scalar1=wt["ba2"][:], scalar2=0.0,
                                            op0=OP.add, op1=OP.max)
                    tg = chk.tile([64, CH], BF16, name="tg",
                                  padded_shape=[64, CHmax])
                    nc.scalar.activation(tg[:], ps6[64:128, :CH], AF.Tanh,
                                         bias=wt["bg2h"][:], scale=0.5)
                    gm = chk.tile([64, CH], BF16, name="gm",
                                  padded_shape=[64, CHmax])
                    nc.vector.scalar_tensor_tensor(
                        out=gm[:], in0=tg[:], scalar=1.0, in1=ps5[:, :CH],
                        op0=OP.add, op1=OP.mult)
                    if q == 1 and g + 1 < NG:
                        tgn = w.tile([82, TGs[g + 1]], BF16,
                                     name=f"tokg_{g + 1}")
                        nc.sync.dma_start(
                            out=tgn[:],
                            in_=t["tok"][:, int(toff[g + 1]):int(toff[g + 2])])
                        toks[g + 1] = tgn

                # ---- tail B of previous chunk: weight + pool (+ group tail)
                if prev is not None:
                    if k < NK:
                        nc.gpsimd.tensor_tensor(out=pmdw[:64, :], in0=pgm[:],
                                                in1=pexpb[:], op=OP.mult)
                    else:
                        nc.vector.tensor_tensor(out=pmdw[:64, :], in0=pgm[:],
                                                in1=ps7[0:64, :pCH], op=OP.mult)
                    mdw5 = pmdw[:].rearrange("p (a r b d) -> p a r b d",
                                             a=1, r=pRC, b=1, d=pD)
                    pqs = slice(pq * pRC, (pq + 1) * pRC)
                    nc.vector.add_instruction(mybir.InstPool(
                        name=f"I-{nc.next_id()}",
                        func=mybir.PoolFunctionType.avg,
                        ins=[nc.vector.lower_ap(mdw5, opt=False)],
                        outs=[nc.vector.lower_ap(msums[pg][:, pqs])]))

                    if pq == RG // pRC - 1:
                        # ---- normalize + combine + output MLP for group pg
                        msum = msums.pop(pg)
                        ztE = grp.tile([65, RG], F32, name="ztE")
                        nc.vector.tensor_scalar_add(out=ztE[64:65, :],
                                                    in0=msum[64:65, :],
                                                    scalar1=1e-30)
                        invzb = grp.tile([65, RG], BF16, name="invzb")
                        with nc.allow_low_precision(reason="denom fits bf16"):
                            nc.vector.reciprocal(out=invzb[64:65, :],
                                                 in_=ztE[64:65, :])
                        psI = pss.tile([64, RG], F32, name="pgrp")
                        nc.tensor.matmul(psI[:], ones64h[64:65, :],
                                         invzb[64:65, :], start=True, stop=True)
                        comb = grp.tile([128, RG], F32, name="comb")
                        gsl = slice(pg * RG, (pg + 1) * RG)
                        nc.scalar.activation(comb[:64, :], selff[:, gsl],
                                             AF.Copy)
                        nc.vector.tensor_tensor(out=comb[64:128, :],
                                                in0=msum[:64, :],
                                                in1=psI[:], op=OP.mult)
                        pc1 = pss.tile([64, RG], F32, name="pgrp")
                        nc.tensor.matmul(pc1[:], wt["Wc1"][:], comb[:],
                                         start=True, stop=True)
                        c1 = grp.tile([64, RG], F32, name="c1")
                        nc.scalar.activation(c1[:], pc1[:], AF.Relu,
                                             bias=wt["bc1"][:])
                        pc2 = pss.tile([64, RG], F32, name="pgrp")
                        nc.tensor.matmul(pc2[:], wt["Wc2"][:], c1[:],
                                         start=True, stop=True)
                        ofm = grp.tile([64, RG], F32, name="ofm")
                        nc.scalar.activation(ofm[:], pc2[:], AF.Identity,
                                             bias=wt["bc2"][:])
                        por = pss.tile([RG, 64], F32, name="pgrp")
                        nc.tensor.transpose(por[:], ofm[:], ident[:64, :64])
                        orow = grp.tile([RG, 64], F32, name="orow")
                        nc.vector.tensor_copy(out=orow[:], in_=por[:])
                        nc.sync.dma_start(out=t["out"][gsl, :], in_=orow[:])

                prev = (h2a, gm, g, q) if k < NK else None
    nc.compile()
    return nc


def _host_prep(x, adjacency, edge_features, weights):
    """Build per-core input maps (pure layout work)."""
    global _DS
    adj = adjacency > 0
    deg = adj.sum(-1)
    order = np.argsort(~adj, axis=-1, kind="stable")   # [B,N,N]

    # per-core degree-sorted row permutation and per-group slot counts
    perms = []
    Ds = np.zeros((NCORES, NG), int)
    for core in range(NCORES):
        b, i0 = core // 4, (core % 4) * RPC
        dcore = deg[b, i0:i0 + RPC]
        perm = np.argsort(-dcore, kind="stable")
        perms.append(perm)
        ds = dcore[perm]
        for g in range(NG):
            Ds[core, g] = max(int(ds[g * RG:(g + 1) * RG].max()), 2)
    DS = tuple(int(((v + 1) // 2) * 2) for v in Ds.max(0))
    _DS = DS
    CHs = [RCH * d for d in DS]
    TGs = [RG * d for d in DS]
    toff = np.concatenate([[0], np.cumsum(TGs)]).astype(int)
    roff = np.concatenate([[0], np.cumsum([NCH * c for c in CHs])]).astype(int)
    T = int(toff[-1])
    CHmax = max(CHs)

    Wa1, Wg1 = weights["Wa1"], weights["Wg1"]
    bhg = np.concatenate([weights["ba1"], weights["bg1"]])
    W22 = np.zeros((128, 128), np.float32)
    W22[:64, :32] = weights["Wa2"]
    W22[64:, 64:] = weights["Wg2"]
    Wa3a = np.concatenate([weights["Wa3"], np.ones((1, 1), np.float32)], 0)
    # R32 block for group g: [p, q*CH_g + n] = 1 iff p % 32 == 8q + n // D_g
    R32 = np.zeros((64, int(roff[-1])), np.float32)
    for g in range(NG):
        pp = np.arange(64)[:, None, None] % 32
        qq = np.arange(NCH)[None, :, None]
        nn = np.arange(CHs[g])[None, None, :] // DS[g]
        R32[:, roff[g]:roff[g + 1]] = (pp == nn + RCH * qq).reshape(
            64, NCH * CHs[g])
    bwts = {
        "We1": weights["We1"], "We2": weights["We2"], "We3": weights["We3"],
        "Wpe": np.concatenate([Wa1[2 * C:], Wg1[C:]], 1),
        "Wjj": np.concatenate([Wa1[C:2 * C], Wg1[:C]], 1),
        "Wn2": weights["Wn"] / 2, "bnT": weights["bn"][None, :] / 2,
        "W22": W22, "Wa3a": Wa3a, "R32": R32,
    }
    bwts = {k: np.ascontiguousarray(v.astype(NPBF)) for k, v in bwts.items()}
    fwts = {
        "Wxi": Wa1[:C], "Ws": weights["Ws"],
        "Wc1": weights["Wc1"], "Wc2": weights["Wc2"],
        "be1": weights["be1"][:, None], "be2": weights["be2"][:, None],
        "be3": weights["be3"][:, None], "ba2": weights["ba2"][:, None],
        "bg2h": weights["bg2"][:, None] / 2, "bhg": bhg[:, None],
        "bs": weights["bs"][:, None],
        "bc1": weights["bc1"][:, None], "bc2": weights["bc2"][:, None],
    }
    fwts = {k: np.ascontiguousarray(v, np.float32) for k, v in fwts.items()}

    in_maps = []
    for core in range(NCORES):
        b, i0 = core // 4, (core % 4) * RPC
        perm = perms[core]
        rows = i0 + perm                              # global node ids, sorted
        m = dict(bwts)
        m.update(fwts)
        tok = np.empty((82, T), NPBF)
        amp = np.zeros((NCHT, CHmax), np.float32)
        for g in range(NG):
            D = DS[g]
            grows = rows[g * RG:(g + 1) * RG]         # [32]
            jr = order[b, grows, :D]                  # [32, D]
            valid = np.arange(D)[None, :] < deg[b, grows][:, None]
            jr = np.where(valid, jr, 0)
            eft = edge_features[b, grows[:, None], jr]   # [32, D, E]
            xjt = x[b][jr]                               # [32, D, C]
            sl = slice(int(toff[g]), int(toff[g + 1]))
            tok[0:64, sl] = xjt.reshape(-1, C).T
            tok[64:82, sl] = eft.reshape(-1, E).T
            av = np.where(valid, 0.0, NEG)               # [32, D]
            amp[g * NCH:(g + 1) * NCH, :RCH * D] = av.reshape(NCH, RCH * D)
        m["tok"] = tok
        m["amp"] = np.ascontiguousarray(amp.astype(NPBF))
        m["xrows"] = np.ascontiguousarray(x[b][rows], np.float32)
        in_maps.append(m)
    return in_maps, perms


def kernel(**inputs):
    x = np.asarray(inputs["x"], np.float32)
    adjacency = np.asarray(inputs["adjacency"], np.float32)
    edge_features = np.asarray(inputs["edge_features"], np.float32)
    weights = {k: np.asarray(v, np.float32) for k, v in inputs.items()
               if k not in ("x", "adjacency", "edge_features")}
    in_maps, perms = _host_prep(x, adjacency, edge_features, weights)
    key = (_DS, 1)
    if key not in _NC:
        _NC[key] = _build_nc()
    res = run_bass_kernel_spmd(_NC[key], in_maps, list(range(NCORES)))
    out = np.zeros((B, N, O), np.float32)
    for core in range(NCORES):
        b, i0 = core // 4, (core % 4) * RPC
        out[b, i0 + perms[core]] = res.results[core]["out"]
    return out


# revision 20
# speedup vs baseline: 1.6239x; 1.6239x over previous
"""Trainium2 Bass kernel for nn_EnhancedGraphConv (gnn_message_passing).

Strategy (8 cores): shard the B*N=1280 graph rows (b,i) as 160 rows/core
(cores 0-3 -> batch 0, 4-7 -> batch 1).  The host sorts each core's rows by
degree and compacts the ~5% active edges into a padded r-major token stream
with a per-group slot count D_g = max degree in that 32-row group (the
padded token count drops ~28% vs a uniform max-degree pad).  Tokens are
pre-gathered and feature-major so the device does zero indirect DMA and
zero layout transposes.  All per-edge MLPs run as bf16 matmuls over
8-row chunks; sigmoid is computed as (1+tanh(x/2))/2 so every scalar-engine
function lives in one act-func set; softmax skips max-subtraction (scores
are O(1)); the adjacency mask enters as an extra contraction row of the
score matmul; exp(score) is broadcast across partitions by an SBUF-to-SBUF
DMA and the message weighting runs on the otherwise-idle gpsimd engine;
per-row reductions are a single windowed avg-pool over a 65-row tile whose
last row carries exp(score), so the softmax denominator falls out of the
same instruction.  The score/exp/broadcast tail of each chunk is
software-pipelined one chunk behind the matmul front.
"""
import numpy as np
from contextlib import ExitStack

import concourse.bass as bass
import concourse.bacc as bacc
import concourse.tile as tile
from concourse import mybir
from concourse.bass_utils import run_bass_kernel_spmd
from concourse.masks import make_identity

F32 = mybir.dt.float32
BF16 = mybir.dt.bfloat16
AF = mybir.ActivationFunctionType
OP = mybir.AluOpType
NPBF = mybir.dt.np(BF16)

B, N, C, O, E = 2, 640, 64, 64, 18
RG = 32           # rows per group
RCH = 8           # rows per chunk
NCORES = 8
RPC = (B * N) // NCORES   # 160 rows per core
NG = RPC // RG            # 5 groups
NCH = RG // RCH           # 4 chunks per group
NCHT = NG * NCH           # 20 chunks

NEG = -1e38

_DS = None        # per-group neighbor slot counts, set by _host_prep
_NC = {}          # compiled kernels keyed by (DS, repeat)


def _build_nc(repeat=1):
    DS = _DS
    RCs = [16 if d <= 32 else RCH for d in DS]
    CHs = [rc * d for rc, d in zip(RCs, DS)]
    TGs = [RG * d for d in DS]
    toff = np.concatenate([[0], np.cumsum(TGs)]).astype(int)
    roff = np.concatenate([[0], np.cumsum(TGs)]).astype(int)
    T = int(toff[-1])
    CHmax = max(CHs)
    chunks = []
    for g in range(NG):
        for q in range(RG // RCs[g]):
            chunks.append((g, q))
    NK = len(chunks)

    nc = bacc.Bacc("TRN2", target_bir_lowering=False)
    t = {}
    bf_inp = [
        ("tok", [82, T]), ("amp", [NCHT, CHmax]), ("R32", [64, T]),
        ("We1", [E, 64]), ("We2", [64, 64]), ("We3", [64, 32]),
        ("Wpe", [32, 128]), ("Wjj", [64, 128]), ("Wn2", [64, 64]),
        ("bnT", [1, 64]), ("W22", [128, 128]), ("Wa3a", [33, 1]),
    ]
    f_inp = [
        ("xrows", [RPC, C]),
        ("Wxi", [64, 64]), ("Ws", [64, 64]), ("Wc1", [128, 64]),
        ("Wc2", [64, 64]),
        ("be1", [64, 1]), ("be2", [64, 1]), ("be3", [32, 1]),
        ("ba2", [32, 1]), ("bg2h", [64, 1]), ("bhg", [128, 1]),
        ("bs", [64, 1]), ("bc1", [64, 1]), ("bc2", [64, 1]),
    ]
    for name, shape in bf_inp:
        t[name] = nc.dram_tensor(name, shape, BF16, kind="ExternalInput")
    for name, shape in f_inp:
        t[name] = nc.dram_tensor(name, shape, F32, kind="ExternalInput")
    t["out"] = nc.dram_tensor("out", [RPC, O], F32, kind="ExternalOutput")

    with tile.TileContext(nc) as tc, ExitStack() as ctx:
        w = ctx.enter_context(tc.tile_pool(name="w", bufs=1))
        chk = ctx.enter_context(tc.tile_pool(name="chk", bufs=4))
        grp = ctx.enter_context(tc.tile_pool(name="grp", bufs=2))
        psc = ctx.enter_context(tc.tile_pool(name="psc", bufs=1, space="PSUM"))
        pss = ctx.enter_context(tc.tile_pool(name="pss", bufs=1, space="PSUM"))

        for _rep in range(repeat):

            ident = w.tile([128, 128], F32)
            make_identity(nc, ident[:])
            ones64h = w.tile([128, 64], BF16)
            nc.vector.memset(ones64h[64:65, :], 1.0)
            onesCH = w.tile([1, CHmax], BF16)
            nc.vector.memset(onesCH[:], 1.0)
            # small loads on the gpsimd DGE queue, streaming tok/R32/xrows
            # on SP, roughly in first-use order
            worder = ["We1", "be1", "Wjj", "Wxi", "We2", "be2",
                      "Wn2", "bnT", "We3", "be3", "Wpe", "bhg", "W22", "ba2",
                      "bg2h", "Wa3a", "Ws", "bs", "Wc1", "bc1", "Wc2", "bc2"]
            wt = {}
            wtE1 = w.tile([128, 64], BF16)
            xr = w.tile([128, 2, C], F32)
            for name in worder:
                if name == "We1":
                    nc.gpsimd.dma_start(out=wtE1[64:64 + E, :], in_=t["We1"][:])
                    continue
                shape = dict(bf_inp + f_inp)[name]
                wt[name] = w.tile(shape, BF16 if dict(bf_inp).get(name) else F32,
                                  name=name)
                nc.gpsimd.dma_start(out=wt[name][:], in_=t[name][:])

            toks0 = []
            for q in range(NCH):
                t0q = w.tile([82, CHs[0]], BF16, name=f"tok0_{q}")
                nc.sync.dma_start(
                    out=t0q[:], in_=t["tok"][:, q * CHs[0]:(q + 1) * CHs[0]])
                toks0.append(t0q)
            nc.sync.dma_start(out=xr[:, 0, :], in_=t["xrows"][0:128, :])
            nc.sync.dma_start(out=xr[:32, 1, :], in_=t["xrows"][128:160, :])
            r32t = w.tile([64, T], BF16)
            nc.sync.dma_start(out=r32t[:], in_=t["R32"][:])
            toks = {0: None}
            tg1 = w.tile([82, TGs[1]], BF16, name="tokg_1")
            nc.sync.dma_start(out=tg1[:],
                              in_=t["tok"][:, int(toff[1]):int(toff[2])])
            toks[1] = tg1

            xrf = w.tile([C, RPC], F32)
            axiT = w.tile([64, 3, C], BF16)
            selff = w.tile([C, RPC], F32)

            def emit_axi_setup():
                p0 = pss.tile([C, RPC], F32, name="pgrp")
                nc.tensor.transpose(p0[:, :128], xr[:, 0, :], ident[:])
                nc.tensor.transpose(p0[:, 128:160], xr[:32, 1, :],
                                    ident[:32, :32])
                nc.vector.tensor_copy(out=xrf[:], in_=p0[:, :RPC])
                pa = pss.tile([C, RPC], F32, name="pgrp")
                nc.tensor.matmul(pa[:], wt["Wxi"][:], xrf[:],
                                 start=True, stop=True)
                axs = w.tile([C, RPC], F32)
                nc.vector.tensor_copy(out=axs[:], in_=pa[:])
                for blk, (c0, c1) in enumerate([(0, 64), (64, 128), (128, 160)]):
                    ptb = pss.tile([64, C], F32, name="pgrp")
                    nc.tensor.transpose(ptb[:c1 - c0, :], axs[:, c0:c1],
                                        ident[:64, :64])
                    nc.vector.tensor_copy(out=axiT[:c1 - c0, blk, :],
                                          in_=ptb[:c1 - c0, :])

            def emit_self_setup():
                pb = pss.tile([C, RPC], F32, name="pgrp")
                nc.tensor.matmul(pb[:], wt["Ws"][:], xrf[:],
                                 start=True, stop=True)
                nc.scalar.activation(selff[:], pb[:], AF.Identity,
                                     bias=wt["bs"][:])

            emit_axi_setup()
            emit_self_setup()

            msums = {}
            prev = None
            for k in range(NK + 1):
                # ---- tail A of previous chunk: score matmul + exp + bcast
                if prev is not None:
                    ph2a, pgm, pg, pq = prev
                    pCH = CHs[pg]
                    pD = DS[pg]
                    pRC = RCs[pg]
                    ps7 = pss.tile([65, CHmax], F32, name="ps7")
                    nc.tensor.matmul(ps7[64:65, :pCH], wt["Wa3a"][:], ph2a[:],
                                     start=True, stop=True)
                    pmdw = chk.tile([65, pCH], BF16, name="mdw",
                                    padded_shape=[65, CHmax])
                    nc.scalar.activation(pmdw[64:65, :], ps7[64:65, :pCH],
                                         AF.Exp)
                    if k < NK:
                        pexpb = chk.tile([64, pCH], BF16, name="pexpb",
                                         padded_shape=[64, CHmax])
                        srow = pmdw[64:65, :]
                        bsrc = bass.AP(tensor=pmdw.tensor, offset=srow.offset,
                                       ap=[srow.ap[0], [0, 64], [1, pCH]])
                        nc.sync.dma_start(out=pexpb[:], in_=bsrc)
                    else:
                        nc.tensor.matmul(ps7[0:64, :pCH], ones64h[64:65, :],
                                         pmdw[64:65, :], start=True, stop=True)

                # ---- front of current chunk
                if k < NK:
                    g, q = chunks[k]
                    D = DS[g]
                    CH = CHs[g]
                    RC = RCs[g]
                    if q == 0:
                        msums[g] = grp.tile([65, RG], F32, name="msum")
                    if g == 0:
                        tkt, c0 = toks0[q], 0
                    else:
                        tkt, c0 = toks[g], q * CH
                    cols = slice(c0, c0 + CH)
                    rb = g * RG + q * RC
                    blk = rb // 64
                    b32 = 32 * ((rb % 64) // 32)
                    r32c = int(roff[g]) + q * CH

                    ps1 = psc.tile([64, CHmax], F32, name="ps1")
                    nc.tensor.matmul(ps1[:, :CH], wtE1[64:64 + E, :],
                                     tkt[64:64 + E, cols], start=True, stop=True)
                    ps4 = psc.tile([128, CHmax], F32, name="ps4")
                    nc.tensor.matmul(ps4[:, :CH], wt["Wjj"][:], tkt[0:64, cols],
                                     start=True, stop=False)
                    nc.tensor.matmul(ps4[:64, :CH], axiT[b32:b32 + 32, blk, :],
                                     r32t[b32:b32 + 32, r32c:r32c + CH],
                                     start=False, stop=False)
                    ps5 = psc.tile([64, CHmax], F32, name="ps5")
                    nc.tensor.matmul(ps5[:, :CH], wt["Wn2"][:], tkt[0:64, cols],
                                     start=True, stop=False)
                    nc.tensor.matmul(ps5[:, :CH], wt["bnT"][:], onesCH[:, :CH],
                                     start=False, stop=True)
                    pe1 = chk.tile([64, CH], BF16, name="pe1",
                                   padded_shape=[64, CHmax])
                    nc.vector.tensor_scalar(out=pe1[:], in0=ps1[:, :CH],
                                            scalar1=wt["be1"][:], scalar2=0.0,
                                            op0=OP.add, op1=OP.max)
                    ps2 = psc.tile([64, CHmax], F32, name="ps2")
                    nc.tensor.matmul(ps2[:, :CH], wt["We2"][:], pe1[:],
                                     start=True, stop=True)
                    pe2 = chk.tile([64, CH], BF16, name="pe2",
                                   padded_shape=[64, CHmax])
                    nc.scalar.activation(pe2[:], ps2[:, :CH], AF.Relu,
                                         bias=wt["be2"][:])
                    ps3 = psc.tile([32, CHmax], F32, name="ps3")
                    nc.tensor.matmul(ps3[:, :CH], wt["We3"][:], pe2[:],
                                     start=True, stop=True)
                    pe3 = chk.tile([32, CH], BF16, name="pe3",
                                   padded_shape=[32, CHmax])
                    if k % 2 == 1:
                        nc.vector.tensor_scalar(out=pe3[:], in0=ps3[:, :CH],
                                                scalar1=wt["be3"][:],
                                                scalar2=0.0,
                                                op0=OP.add, op1=OP.max)
                    else:
                        nc.scalar.activation(pe3[:], ps3[:, :CH], AF.Relu,
                                             bias=wt["be3"][:])
                    nc.tensor.matmul(ps4[:, :CH], wt["Wpe"][:], pe3[:],
                                     start=False, stop=True)
                    hg = chk.tile([128, CH], BF16, name="hg",
                                  padded_shape=[128, CHmax])
                    nc.scalar.activation(hg[:], ps4[:, :CH], AF.Relu,
                                         bias=wt["bhg"][:])
                    ps6 = psc.tile([128, CHmax], F32, name="ps6")
                    nc.tensor.matmul(ps6[:, :CH], wt["W22"][:], hg[:],
                                     start=True, stop=True)
                    h2a = chk.tile([33, CH], BF16, name="h2a",
                                   padded_shape=[33, CHmax])
                    nc.sync.dma_start(out=h2a[32:33, :],
                                      in_=t["amp"][k:k + 1, 0:CH])
                    nc.vector.tensor_scalar(out=h2a[:32, :], in0=ps6[:32, :CH],
                                            scalar1=wt["ba2"][:], scalar2=0.0,
                                            op0=OP.add, op1=OP.max)
                    tg = chk.tile([64, CH], BF16, name="tg",
                                  padded_shape=[64, CHmax])
                    nc.scalar.activation(tg[:], ps6[64:128, :CH], AF.Tanh,
                                         bias=wt["bg2h"][:], scale=0.5)
                    gm = chk.tile([64, CH], BF16, name="gm",
                                  padded_shape=[64, CHmax])
                    nc.vector.scalar_tensor_tensor(
                        out=gm[:], in0=tg[:], scalar=1.0, in1=ps5[:, :CH],
                        op0=OP.add, op1=OP.mult)
                    if q == 1 and g + 1 < NG:
                        tgn = w.tile([82, TGs[g + 1]], BF16,
                                     name=f"tokg_{g + 1}")
                        nc.sync.dma_start(
                            out=tgn[:],
                            in_=t["tok"][:, int(toff[g + 1]):int(toff[g + 2])])
                        toks[g + 1] = tgn

                # ---- tail B of previous chunk: weight + pool (+ group tail)
                if prev is not None:
                    if k < NK:
                        nc.gpsimd.tensor_tensor(out=pmdw[:64, :], in0=pgm[:],
                                                in1=pexpb[:], op=OP.mult)
                    else:
                        nc.vector.tensor_tensor(out=pmdw[:64, :], in0=pgm[:],
                                                in1=ps7[0:64, :pCH], op=OP.mult)
                    mdw5 = pmdw[:].rearrange("p (a r b d) -> p a r b d",
                                             a=1, r=pRC, b=1, d=pD)
                    pqs = slice(pq * pRC, (pq + 1) * pRC)
                    nc.vector.add_instruction(mybir.InstPool(
                        name=f"I-{nc.next_id()}",
                        func=mybir.PoolFunctionType.avg,
                        ins=[nc.vector.lower_ap(mdw5, opt=False)],
                        outs=[nc.vector.lower_ap(msums[pg][:, pqs])]))

                    if pq == RG // pRC - 1:
                        # ---- normalize + combine + output MLP for group pg
                        msum = msums.pop(pg)
                        ztE = grp.tile([65, RG], F32, name="ztE")
                        nc.vector.tensor_scalar_add(out=ztE[64:65, :],
                                                    in0=msum[64:65, :],
                                                    scalar1=1e-30)
                        invzb = grp.tile([65, RG], BF16, name="invzb")
                        with nc.allow_low_precision(reason="denom fits bf16"):
                            nc.vector.reciprocal(out=invzb[64:65, :],
                                                 in_=ztE[64:65, :])
                        psI = pss.tile([64, RG], F32, name="pgrp")
                        nc.tensor.matmul(psI[:], ones64h[64:65, :],
                                         invzb[64:65, :], start=True, stop=True)
                        comb = grp.tile([128, RG], F32, name="comb")
                        gsl = slice(pg * RG, (pg + 1) * RG)
                        nc.scalar.activation(comb[:64, :], selff[:, gsl],
                                             AF.Copy)
                        nc.vector.tensor_tensor(out=comb[64:128, :],
                                                in0=msum[:64, :],
                                                in1=psI[:], op=OP.mult)
                        pc1 = pss.tile([64, RG], F32, name="pgrp")
                        nc.tensor.matmul(pc1[:], wt["Wc1"][:], comb[:],
                                         start=True, stop=True)
                        c1 = grp.tile([64, RG], F32, name="c1")
                        nc.scalar.activation(c1[:], pc1[:], AF.Relu,
                                             bias=wt["bc1"][:])
                        pc2 = pss.tile([64, RG], F32, name="pgrp")
                        nc.tensor.matmul(pc2[:], wt["Wc2"][:], c1[:],
                                         start=True, stop=True)
                        ofm = grp.tile([64, RG], F32, name="ofm")
                        nc.scalar.activation(ofm[:], pc2[:], AF.Identity,
                                             bias=wt["bc2"][:])
                        por = pss.tile([RG, 64], F32, name="pgrp")
                        nc.tensor.transpose(por[:], ofm[:], ident[:64, :64])
                        orow = grp.tile([RG, 64], F32, name="orow")
                        nc.vector.tensor_copy(out=orow[:], in_=por[:])
                        nc.sync.dma_start(out=t["out"][gsl, :], in_=orow[:])

                prev = (h2a, gm, g, q) if k < NK else None
    nc.compile()
    return nc


def _host_prep(x, adjacency, edge_features, weights):
    """Build per-core input maps (pure layout work)."""
    global _DS
    adj = adjacency > 0
    deg = adj.sum(-1)
    order = np.argsort(~adj, axis=-1, kind="stable")   # [B,N,N]

    # per-core degree-sorted row permutation and per-group slot counts
    perms = []
    Ds = np.zeros((NCORES, NG), int)
    for core in range(NCORES):
        b, i0 = core // 4, (core % 4) * RPC
        dcore = deg[b, i0:i0 + RPC]
        perm = np.argsort(-dcore, kind="stable")
        perms.append(perm)
        ds = dcore[perm]
        for g in range(NG):
            Ds[core, g] = max(int(ds[g * RG:(g + 1) * RG].max()), 2)
    DS = tuple(int(((v + 1) // 2) * 2) for v in Ds.max(0))
    _DS = DS
    RCs = [16 if d <= 32 else RCH for d in DS]
    CHs = [rc * d for rc, d in zip(RCs, DS)]
    TGs = [RG * d for d in DS]
    toff = np.concatenate([[0], np.cumsum(TGs)]).astype(int)
    T = int(toff[-1])
    CHmax = max(CHs)

    Wa1, Wg1 = weights["Wa1"], weights["Wg1"]
    bhg = np.concatenate([weights["ba1"], weights["bg1"]])
    W22 = np.zeros((128, 128), np.float32)
    W22[:64, :32] = weights["Wa2"]
    W22[64:, 64:] = weights["Wg2"]
    Wa3a = np.concatenate([weights["Wa3"], np.ones((1, 1), np.float32)], 0)
    # R32 block for group g: [p, q*CH_g + n] = 1 iff p % 32 == RC_g*q + n//D_g
    R32 = np.zeros((64, T), np.float32)
    for g in range(NG):
        nq = RG // RCs[g]
        pp = np.arange(64)[:, None, None] % 32
        qq = np.arange(nq)[None, :, None]
        nn = np.arange(CHs[g])[None, None, :] // DS[g]
        R32[:, toff[g]:toff[g + 1]] = (pp == nn + RCs[g] * qq).reshape(
            64, nq * CHs[g])
    bwts = {
        "We1": weights["We1"], "We2": weights["We2"], "We3": weights["We3"],
        "Wpe": np.concatenate([Wa1[2 * C:], Wg1[C:]], 1),
        "Wjj": np.concatenate([Wa1[C:2 * C], Wg1[:C]], 1),
        "Wn2": weights["Wn"] / 2, "bnT": weights["bn"][None, :] / 2,
        "W22": W22, "Wa3a": Wa3a, "R32": R32,
    }
    bwts = {k: np.ascontiguousarray(v.astype(NPBF)) for k, v in bwts.items()}
    fwts = {
        "Wxi": Wa1[:C], "Ws": weights["Ws"],
        "Wc1": weights["Wc1"], "Wc2": weights["Wc2"],
        "be1": weights["be1"][:, None], "be2": weights["be2"][:, None],
        "be3": weights["be3"][:, None], "ba2": weights["ba2"][:, None],
        "bg2h": weights["bg2"][:, None] / 2, "bhg": bhg[:, None],
        "bs": weights["bs"][:, None],
        "bc1": weights["bc1"][:, None], "bc2": weights["bc2"][:, None],
    }
    fwts = {k: np.ascontiguousarray(v, np.float32) for k, v in fwts.items()}

    in_maps = []
    for core in range(NCORES):
        b, i0 = core // 4, (core % 4) * RPC
        perm = perms[core]
        rows = i0 + perm                              # global node ids, sorted
        m = dict(bwts)
        m.update(fwts)
        tok = np.empty((82, T), NPBF)
        amp = np.zeros((NCHT, CHmax), np.float32)
        krow = 0
        for g in range(NG):
            D = DS[g]
            nq = RG // RCs[g]
            grows = rows[g * RG:(g + 1) * RG]         # [32]
            jr = order[b, grows, :D]                  # [32, D]
            valid = np.arange(D)[None, :] < deg[b, grows][:, None]
            jr = np.where(valid, jr, 0)
            eft = edge_features[b, grows[:, None], jr]   # [32, D, E]
            xjt = x[b][jr]                               # [32, D, C]
            sl = slice(int(toff[g]), int(toff[g + 1]))
            tok[0:64, sl] = xjt.reshape(-1, C).T
            tok[64:82, sl] = eft.reshape(-1, E).T
            av = np.where(valid, 0.0, NEG)               # [32, D]
            amp[krow:krow + nq, :RCs[g] * D] = av.reshape(nq, RCs[g] * D)
            krow += nq
        m["tok"] = tok
        m["amp"] = np.ascontiguousarray(amp.astype(NPBF))
        m["xrows"] = np.ascontiguousarray(x[b][rows], np.float32)
        in_maps.append(m)
    return in_maps, perms


def kernel(**inputs):
    x = np.asarray(inputs["x"], np.float32)
    adjacency = np.asarray(inputs["adjacency"], np.float32)
    edge_features = np.asarray(inputs["edge_features"], np.float32)
    weights = {k: np.asarray(v, np.float32) for k, v in inputs.items()
               if k not in ("x", "adjacency", "edge_features")}
    in_maps, perms = _host_prep(x, adjacency, edge_features, weights)
    key = (_DS, 1)
    if key not in _NC:
        _NC[key] = _build_nc()
    res = run_bass_kernel_spmd(_NC[key], in_maps, list(range(NCORES)))
    out = np.zeros((B, N, O), np.float32)
    for core in range(NCORES):
        b, i0 = core // 4, (core % 4) * RPC
        out[b, i0 + perms[core]] = res.results[core]["out"]
    return out


# revision 21
# speedup vs baseline: 1.6595x; 1.0219x over previous
"""Trainium2 Bass kernel for nn_EnhancedGraphConv (gnn_message_passing).

Strategy (8 cores): shard the B*N=1280 graph rows (b,i) as 160 rows/core
(cores 0-3 -> batch 0, 4-7 -> batch 1).  The host sorts each core's rows by
degree and compacts the ~5% active edges into a padded r-major token stream
with a per-group slot count D_g = max degree in that 32-row group (the
padded token count drops ~28% vs a uniform max-degree pad).  Tokens are
pre-gathered and feature-major so the device does zero indirect DMA and
zero layout transposes.  All per-edge MLPs run as bf16 matmuls over
8-row chunks; sigmoid is computed as (1+tanh(x/2))/2 so every scalar-engine
function lives in one act-func set; softmax skips max-subtraction (scores
are O(1)); the adjacency mask enters as an extra contraction row of the
score matmul; exp(score) is broadcast across partitions by an SBUF-to-SBUF
DMA and the message weighting runs on the otherwise-idle gpsimd engine;
per-row reductions are a single windowed avg-pool over a 65-row tile whose
last row carries exp(score), so the softmax denominator falls out of the
same instruction.  The score/exp/broadcast tail of each chunk is
software-pipelined one chunk behind the matmul front.
"""
import numpy as np
from contextlib import ExitStack

import concourse.bass as bass
import concourse.bacc as bacc
import concourse.tile as tile
from concourse import mybir
from concourse.bass_utils import run_bass_kernel_spmd
from concourse.masks import make_identity

F32 = mybir.dt.float32
BF16 = mybir.dt.bfloat16
AF = mybir.ActivationFunctionType
OP = mybir.AluOpType
NPBF = mybir.dt.np(BF16)

B, N, C, O, E = 2, 640, 64, 64, 18
RG = 32           # rows per group
RCH = 8           # rows per chunk
NCORES = 8
RPC = (B * N) // NCORES   # 160 rows per core
NG = RPC // RG            # 5 groups
NCH = RG // RCH           # 4 chunks per group
NCHT = NG * NCH           # 20 chunks

NEG = -1e38

_DS = None        # per-group neighbor slot counts, set by _host_prep
_NC = {}          # compiled kernels keyed by (DS, repeat)


def _build_nc(repeat=1):
    DS = _DS
    RCs = [16 if d <= 32 else RCH for d in DS]
    CHs = [rc * d for rc, d in zip(RCs, DS)]
    TGs = [RG * d for d in DS]
    toff = np.concatenate([[0], np.cumsum(TGs)]).astype(int)
    roff = np.concatenate([[0], np.cumsum(TGs)]).astype(int)
    T = int(toff[-1])
    CHmax = max(CHs)
    chunks = []
    for g in range(NG):
        for q in range(RG // RCs[g]):
            chunks.append((g, q))
    NK = len(chunks)

    nc = bacc.Bacc("TRN2", target_bir_lowering=False)
    t = {}
    bf_inp = [
        ("tok", [82, T]), ("amp", [NCHT, CHmax]), ("R32", [64, T]),
        ("We1", [E, 64]), ("We2", [64, 64]), ("We3", [64, 32]),
        ("Wpe", [32, 128]), ("Wjj", [64, 128]), ("Wn2", [64, 64]),
        ("bnT", [1, 64]), ("W22", [128, 128]), ("Wa3a", [33, 1]),
    ]
    f_inp = [
        ("xrows", [RPC, C]),
        ("Wxi", [64, 64]), ("Ws", [64, 64]), ("Wc1", [128, 64]),
        ("Wc2", [64, 64]),
        ("be1", [64, 1]), ("be2", [64, 1]), ("be3", [32, 1]),
        ("ba2", [32, 1]), ("bg2h", [64, 1]), ("bhg", [128, 1]),
        ("bs", [64, 1]), ("bc1", [64, 1]), ("bc2", [64, 1]),
    ]
    for name, shape in bf_inp:
        t[name] = nc.dram_tensor(name, shape, BF16, kind="ExternalInput")
    for name, shape in f_inp:
        t[name] = nc.dram_tensor(name, shape, F32, kind="ExternalInput")
    t["out"] = nc.dram_tensor("out", [RPC, O], F32, kind="ExternalOutput")

    with tile.TileContext(nc) as tc, ExitStack() as ctx:
        w = ctx.enter_context(tc.tile_pool(name="w", bufs=1))
        chk = ctx.enter_context(tc.tile_pool(name="chk", bufs=4))
        grp = ctx.enter_context(tc.tile_pool(name="grp", bufs=2))
        psc = ctx.enter_context(tc.tile_pool(name="psc", bufs=1, space="PSUM"))
        pss = ctx.enter_context(tc.tile_pool(name="pss", bufs=1, space="PSUM"))

        for _rep in range(repeat):

            ident = w.tile([128, 128], F32)
            make_identity(nc, ident[:])
            ones64h = w.tile([128, 64], BF16)
            nc.vector.memset(ones64h[64:65, :], 1.0)
            onesCH = w.tile([1, CHmax], BF16)
            nc.vector.memset(onesCH[:], 1.0)
            # small loads on the gpsimd DGE queue, streaming tok/R32/xrows
            # on SP, roughly in first-use order
            worder = ["We1", "be1", "Wjj", "Wxi", "We2", "be2",
                      "Wn2", "bnT", "We3", "be3", "Wpe", "bhg", "W22", "ba2",
                      "bg2h", "Wa3a", "Ws", "bs", "Wc1", "bc1", "Wc2", "bc2"]
            wt = {}
            wtE1 = w.tile([128, 64], BF16)
            xr = w.tile([128, 2, C], F32)
            for name in worder:
                if name == "We1":
                    nc.gpsimd.dma_start(out=wtE1[64:64 + E, :], in_=t["We1"][:])
                    continue
                shape = dict(bf_inp + f_inp)[name]
                wt[name] = w.tile(shape, BF16 if dict(bf_inp).get(name) else F32,
                                  name=name)
                nc.gpsimd.dma_start(out=wt[name][:], in_=t[name][:])

            toks0 = []
            for q in range(NCH):
                t0q = w.tile([82, CHs[0]], BF16, name=f"tok0_{q}")
                nc.sync.dma_start(
                    out=t0q[:], in_=t["tok"][:, q * CHs[0]:(q + 1) * CHs[0]])
                toks0.append(t0q)
            nc.sync.dma_start(out=xr[:, 0, :], in_=t["xrows"][0:128, :])
            nc.sync.dma_start(out=xr[:32, 1, :], in_=t["xrows"][128:160, :])
            r32t = w.tile([64, T], BF16)
            nc.sync.dma_start(out=r32t[:], in_=t["R32"][:])
            toks = {0: None}
            tg1 = w.tile([82, TGs[1]], BF16, name="tokg_1")
            nc.sync.dma_start(out=tg1[:],
                              in_=t["tok"][:, int(toff[1]):int(toff[2])])
            toks[1] = tg1

            xrf = w.tile([C, RPC], F32)
            axiT = w.tile([64, 3, C], BF16)
            selff = w.tile([C, RPC], F32)

            def emit_axi_setup():
                p0 = pss.tile([C, RPC], F32, name="pgrp")
                nc.tensor.transpose(p0[:, :128], xr[:, 0, :], ident[:])
                nc.tensor.transpose(p0[:, 128:160], xr[:32, 1, :],
                                    ident[:32, :32])
                nc.vector.tensor_copy(out=xrf[:], in_=p0[:, :RPC])
                pa = pss.tile([C, RPC], F32, name="pgrp")
                nc.tensor.matmul(pa[:], wt["Wxi"][:], xrf[:],
                                 start=True, stop=True)
                axs = w.tile([C, RPC], F32)
                nc.vector.tensor_copy(out=axs[:], in_=pa[:])
                for blk, (c0, c1) in enumerate([(0, 64), (64, 128), (128, 160)]):
                    ptb = pss.tile([64, C], F32, name="pgrp")
                    nc.tensor.transpose(ptb[:c1 - c0, :], axs[:, c0:c1],
                                        ident[:64, :64])
                    nc.vector.tensor_copy(out=axiT[:c1 - c0, blk, :],
                                          in_=ptb[:c1 - c0, :])

            def emit_self_setup():
                pb = pss.tile([C, RPC], F32, name="pgrp")
                nc.tensor.matmul(pb[:], wt["Ws"][:], xrf[:],
                                 start=True, stop=True)
                nc.scalar.activation(selff[:], pb[:], AF.Identity,
                                     bias=wt["bs"][:])

            emit_axi_setup()
            emit_self_setup()

            msums = {}
            prev = None
            for k in range(NK + 1):
                # ---- tail A of previous chunk: score matmul + exp + bcast
                if prev is not None:
                    ph2a, pgm, pg, pq = prev
                    pCH = CHs[pg]
                    pD = DS[pg]
                    pRC = RCs[pg]
                    ps7 = pss.tile([65, CHmax], F32, name="ps7")
                    nc.tensor.matmul(ps7[64:65, :pCH], wt["Wa3a"][:], ph2a[:],
                                     start=True, stop=True)
                    pmdw = chk.tile([65, pCH], BF16, name="mdw",
                                    padded_shape=[65, CHmax])
                    nc.scalar.activation(pmdw[64:65, :], ps7[64:65, :pCH],
                                         AF.Exp)
                    if k < NK:
                        pexpb = chk.tile([64, pCH], BF16, name="pexpb",
                                         padded_shape=[64, CHmax])
                        srow = pmdw[64:65, :]
                        bsrc = bass.AP(tensor=pmdw.tensor, offset=srow.offset,
                                       ap=[srow.ap[0], [0, 64], [1, pCH]])
                        nc.sync.dma_start(out=pexpb[:], in_=bsrc)
                    else:
                        nc.tensor.matmul(ps7[0:64, :pCH], ones64h[64:65, :],
                                         pmdw[64:65, :], start=True, stop=True)

                # ---- front of current chunk
                if k < NK:
                    g, q = chunks[k]
                    D = DS[g]
                    CH = CHs[g]
                    RC = RCs[g]
                    if q == 0:
                        msums[g] = grp.tile([65, RG], F32, name="msum")
                    if g == 0:
                        tkt, c0 = toks0[q], 0
                    else:
                        tkt, c0 = toks[g], q * CH
                    cols = slice(c0, c0 + CH)
                    rb = g * RG + q * RC
                    blk = rb // 64
                    b32 = 32 * ((rb % 64) // 32)
                    r32c = int(roff[g]) + q * CH

                    ps1 = psc.tile([64, CHmax], F32, name="ps1")
                    nc.tensor.matmul(ps1[:, :CH], wtE1[64:64 + E, :],
                                     tkt[64:64 + E, cols], start=True, stop=True)
                    ps4 = psc.tile([128, CHmax], F32, name="ps4")
                    nc.tensor.matmul(ps4[:, :CH], wt["Wjj"][:], tkt[0:64, cols],
                                     start=True, stop=False)
                    nc.tensor.matmul(ps4[:64, :CH], axiT[b32:b32 + 32, blk, :],
                                     r32t[b32:b32 + 32, r32c:r32c + CH],
                                     start=False, stop=False)
                    ps5 = psc.tile([64, CHmax], F32, name="ps5")
                    nc.tensor.matmul(ps5[:, :CH], wt["Wn2"][:], tkt[0:64, cols],
                                     start=True, stop=False)
                    nc.tensor.matmul(ps5[:, :CH], wt["bnT"][:], onesCH[:, :CH],
                                     start=False, stop=True)
                    pe1 = chk.tile([64, CH], BF16, name="pe1",
                                   padded_shape=[64, CHmax])
                    nc.vector.tensor_scalar(out=pe1[:], in0=ps1[:, :CH],
                                            scalar1=wt["be1"][:], scalar2=0.0,
                                            op0=OP.add, op1=OP.max)
                    ps2 = psc.tile([64, CHmax], F32, name="ps2")
                    nc.tensor.matmul(ps2[:, :CH], wt["We2"][:], pe1[:],
                                     start=True, stop=True)
                    pe2 = chk.tile([64, CH], BF16, name="pe2",
                                   padded_shape=[64, CHmax])
                    nc.scalar.activation(pe2[:], ps2[:, :CH], AF.Relu,
                                         bias=wt["be2"][:])
                    ps3 = psc.tile([32, CHmax], F32, name="ps3")
                    nc.tensor.matmul(ps3[:, :CH], wt["We3"][:], pe2[:],
                                     start=True, stop=True)
                    pe3 = chk.tile([32, CH], BF16, name="pe3",
                                   padded_shape=[32, CHmax])
                    if k % 2 == 1:
                        nc.vector.tensor_scalar(out=pe3[:], in0=ps3[:, :CH],
                                                scalar1=wt["be3"][:],
                                                scalar2=0.0,
                                                op0=OP.add, op1=OP.max)
                    else:
                        nc.scalar.activation(pe3[:], ps3[:, :CH], AF.Relu,
                                             bias=wt["be3"][:])
                    nc.tensor.matmul(ps4[:, :CH], wt["Wpe"][:], pe3[:],
                                     start=False, stop=True)
                    hg = chk.tile([128, CH], BF16, name="hg",
                                  padded_shape=[128, CHmax])
                    nc.scalar.activation(hg[:], ps4[:, :CH], AF.Relu,
                                         bias=wt["bhg"][:])
                    ps6 = psc.tile([128, CHmax], F32, name="ps6")
                    nc.tensor.matmul(ps6[:, :CH], wt["W22"][:], hg[:],
                                     start=True, stop=True)
                    h2a = chk.tile([33, CH], BF16, name="h2a",
                                   padded_shape=[33, CHmax])
                    nc.sync.dma_start(out=h2a[32:33, :],
                                      in_=t["amp"][k:k + 1, 0:CH])
                    nc.vector.tensor_scalar(out=h2a[:32, :], in0=ps6[:32, :CH],
                                            scalar1=wt["ba2"][:], scalar2=0.0,
                                            op0=OP.add, op1=OP.max)
                    tg = chk.tile([64, CH], BF16, name="tg",
                                  padded_shape=[64, CHmax])
                    nc.scalar.activation(tg[:], ps6[64:128, :CH], AF.Tanh,
                                         bias=wt["bg2h"][:], scale=0.5)
                    gm = chk.tile([64, CH], BF16, name="gm",
                                  padded_shape=[64, CHmax])
                    nc.vector.scalar_tensor_tensor(
                        out=gm[:], in0=tg[:], scalar=1.0, in1=ps5[:, :CH],
                        op0=OP.add, op1=OP.mult)
                    if q == 1 and g + 1 < NG:
                        tgn = w.tile([82, TGs[g + 1]], BF16,
                                     name=f"tokg_{g + 1}")
                        nc.sync.dma_start(
                            out=tgn[:],
                            in_=t["tok"][:, int(toff[g + 1]):int(toff[g + 2])])
                        toks[g + 1] = tgn

                # ---- tail B of previous chunk: weight + pool (+ group tail)
                if prev is not None:
                    if k < NK:
                        nc.gpsimd.tensor_tensor(out=pmdw[:64, :], in0=pgm[:],
                                                in1=pexpb[:], op=OP.mult)
                    else:
                        nc.vector.tensor_tensor(out=pmdw[:64, :], in0=pgm[:],
                                                in1=ps7[0:64, :pCH], op=OP.mult)
                    mdw5 = pmdw[:].rearrange("p (a r b d) -> p a r b d",
                                             a=1, r=pRC, b=1, d=pD)
                    pqs = slice(pq * pRC, (pq + 1) * pRC)
                    nc.vector.add_instruction(mybir.InstPool(
                        name=f"I-{nc.next_id()}",
                        func=mybir.PoolFunctionType.avg,
                        ins=[nc.vector.lower_ap(mdw5, opt=False)],
                        outs=[nc.vector.lower_ap(msums[pg][:, pqs])]))

                    if pq == RG // pRC - 1:
                        # ---- normalize + combine + output MLP for group pg
                        msum = msums.pop(pg)
                        ztE = grp.tile([65, RG], F32, name="ztE")
                        nc.vector.tensor_scalar_add(out=ztE[64:65, :],
                                                    in0=msum[64:65, :],
                                                    scalar1=1e-30)
                        invzb = grp.tile([65, RG], BF16, name="invzb")
                        with nc.allow_low_precision(reason="denom fits bf16"):
                            nc.vector.reciprocal(out=invzb[64:65, :],
                                                 in_=ztE[64:65, :])
                        psI = pss.tile([64, RG], F32, name="pgrp")
                        nc.tensor.matmul(psI[:], ones64h[64:65, :],
                                         invzb[64:65, :], start=True, stop=True)
                        comb = grp.tile([128, RG], F32, name="comb")
                        gsl = slice(pg * RG, (pg + 1) * RG)
                        nc.scalar.activation(comb[:64, :], selff[:, gsl],
                                             AF.Copy)
                        nc.vector.tensor_tensor(out=comb[64:128, :],
                                                in0=msum[:64, :],
                                                in1=psI[:], op=OP.mult)
                        pc1 = pss.tile([64, RG], F32, name="pgrp")
                        nc.tensor.matmul(pc1[:], wt["Wc1"][:], comb[:],
                                         start=True, stop=True)
                        c1 = grp.tile([64, RG], F32, name="c1")
                        nc.scalar.activation(c1[:], pc1[:], AF.Relu,
                                             bias=wt["bc1"][:])
                        pc2 = pss.tile([64, RG], F32, name="pgrp")
                        nc.tensor.matmul(pc2[:], wt["Wc2"][:], c1[:],
                                         start=True, stop=True)
                        ofm = grp.tile([64, RG], F32, name="ofm")
                        nc.scalar.activation(ofm[:], pc2[:], AF.Identity,
                                             bias=wt["bc2"][:])
                        por = pss.tile([RG, 64], F32, name="pgrp")
                        nc.tensor.transpose(por[:], ofm[:], ident[:64, :64])
                        orow = grp.tile([RG, 64], F32, name="orow")
                        nc.vector.tensor_copy(out=orow[:], in_=por[:])
                        nc.sync.dma_start(out=t["out"][gsl, :], in_=orow[:])

                prev = (h2a, gm, g, q) if k < NK else None
    nc.compile()
    return nc


def _host_prep(x, adjacency, edge_features, weights):
    """Build per-core input maps (pure layout work)."""
    global _DS
    adj = adjacency > 0
    deg = adj.sum(-1)
    order = np.argsort(~adj, axis=-1, kind="stable")   # [B,N,N]

    # per-core degree-sorted row permutation and per-group slot counts
    perms = []
    Ds = np.zeros((NCORES, NG), int)
    for core in range(NCORES):
        b, i0 = core // 4, (core % 4) * RPC
        dcore = deg[b, i0:i0 + RPC]
        perm = np.argsort(-dcore, kind="stable")
        perms.append(perm)
        ds = dcore[perm]
        for g in range(NG):
            Ds[core, g] = max(int(ds[g * RG:(g + 1) * RG].max()), 2)
    DS = tuple(int(((v + 1) // 2) * 2) for v in Ds.max(0))
    assert DS[0] <= 64, f"max degree {DS[0]} exceeds 64 slots/row"
    _DS = DS
    RCs = [16 if d <= 32 else RCH for d in DS]
    CHs = [rc * d for rc, d in zip(RCs, DS)]
    TGs = [RG * d for d in DS]
    toff = np.concatenate([[0], np.cumsum(TGs)]).astype(int)
    T = int(toff[-1])
    CHmax = max(CHs)

    Wa1, Wg1 = weights["Wa1"], weights["Wg1"]
    bhg = np.concatenate([weights["ba1"], weights["bg1"]])
    W22 = np.zeros((128, 128), np.float32)
    W22[:64, :32] = weights["Wa2"]
    W22[64:, 64:] = weights["Wg2"]
    Wa3a = np.concatenate([weights["Wa3"], np.ones((1, 1), np.float32)], 0)
    # R32 block for group g: [p, q*CH_g + n] = 1 iff p % 32 == RC_g*q + n//D_g
    R32 = np.zeros((64, T), np.float32)
    for g in range(NG):
        nq = RG // RCs[g]
        pp = np.arange(64)[:, None, None] % 32
        qq = np.arange(nq)[None, :, None]
        nn = np.arange(CHs[g])[None, None, :] // DS[g]
        R32[:, toff[g]:toff[g + 1]] = (pp == nn + RCs[g] * qq).reshape(
            64, nq * CHs[g])
    bwts = {
        "We1": weights["We1"], "We2": weights["We2"], "We3": weights["We3"],
        "Wpe": np.concatenate([Wa1[2 * C:], Wg1[C:]], 1),
        "Wjj": np.concatenate([Wa1[C:2 * C], Wg1[:C]], 1),
        "Wn2": weights["Wn"] / 2, "bnT": weights["bn"][None, :] / 2,
        "W22": W22, "Wa3a": Wa3a, "R32": R32,
    }
    bwts = {k: np.ascontiguousarray(v.astype(NPBF)) for k, v in bwts.items()}
    fwts = {
        "Wxi": Wa1[:C], "Ws": weights["Ws"],
        "Wc1": weights["Wc1"], "Wc2": weights["Wc2"],
        "be1": weights["be1"][:, None], "be2": weights["be2"][:, None],
        "be3": weights["be3"][:, None], "ba2": weights["ba2"][:, None],
        "bg2h": weights["bg2"][:, None] / 2, "bhg": bhg[:, None],
        "bs": weights["bs"][:, None],
        "bc1": weights["bc1"][:, None], "bc2": weights["bc2"][:, None],
    }
    fwts = {k: np.ascontiguousarray(v, np.float32) for k, v in fwts.items()}

    in_maps = []
    for core in range(NCORES):
        b, i0 = core // 4, (core % 4) * RPC
        perm = perms[core]
        rows = i0 + perm                              # global node ids, sorted
        m = dict(bwts)
        m.update(fwts)
        tok = np.empty((82, T), NPBF)
        amp = np.zeros((NCHT, CHmax), np.float32)
        krow = 0
        for g in range(NG):
            D = DS[g]
            nq = RG // RCs[g]
            grows = rows[g * RG:(g + 1) * RG]         # [32]
            jr = order[b, grows, :D]                  # [32, D]
            valid = np.arange(D)[None, :] < deg[b, grows][:, None]
            jr = np.where(valid, jr, 0)
            eft = edge_features[b, grows[:, None], jr]   # [32, D, E]
            xjt = x[b][jr]                               # [32, D, C]
            sl = slice(int(toff[g]), int(toff[g + 1]))
            tok[0:64, sl] = xjt.reshape(-1, C).T
            tok[64:82, sl] = eft.reshape(-1, E).T
            av = np.where(valid, 0.0, NEG)               # [32, D]
            amp[krow:krow + nq, :RCs[g] * D] = av.reshape(nq, RCs[g] * D)
            krow += nq
        m["tok"] = tok
        m["amp"] = np.ascontiguousarray(amp.astype(NPBF))
        m["xrows"] = np.ascontiguousarray(x[b][rows], np.float32)
        in_maps.append(m)
    return in_maps, perms


def kernel(**inputs):
    x = np.asarray(inputs["x"], np.float32)
    adjacency = np.asarray(inputs["adjacency"], np.float32)
    edge_features = np.asarray(inputs["edge_features"], np.float32)
    weights = {k: np.asarray(v, np.float32) for k, v in inputs.items()
               if k not in ("x", "adjacency", "edge_features")}
    in_maps, perms = _host_prep(x, adjacency, edge_features, weights)
    key = (_DS, 1)
    if key not in _NC:
        _NC[key] = _build_nc()
    res = run_bass_kernel_spmd(_NC[key], in_maps, list(range(NCORES)))
    out = np.zeros((B, N, O), np.float32)
    for core in range(NCORES):
        b, i0 = core // 4, (core % 4) * RPC
        out[b, i0 + perms[core]] = res.results[core]["out"]
    return out
